# revision 25
# baseline (speedup 1.0000x reference)
"""AdaDRO loss kernel for 8 TRN2 NeuronCores (Bass/Tile, SPMD).

Sharding: N=2048 ptr rows split across 8 cores (256 each); per-nu work
(feature row-normalize, CE losses) sharded by M/8=512 and all-gathered.
classifier_weights are row-normalized to bf16 once per core; label
embeddings are fetched with dma_gather(transpose=True) directly into the
[d%128, d//128, j] matmul layout.  Both cosine GEMMs plus a bf16 rank-3
term (hi/lo split of the CE-loss column + constant) accumulate into one
PSUM tile per output block, so a single Exp activation computes the
softmax numerator with row sums accumulated for free.  Collectives: an
early AllGather for the normalized nu shard, a tiny AllGather for the CE
losses/max, and a final 17KB AllReduce for the worst-case probs.
"""
import numpy as np

N, M, D, K = 2048, 4096, 512, 1000
R = 8
NL = N // R        # 256 ptr rows per core
ML = M // R        # 512 nu rows per core
DC = D // 128      # 4 d-chunks
WC = 1024 // 128   # 8 w row chunks (K padded 1000 -> 1024)
LAMBDA_DRO = 0.1
EPSILON = 0.5
EPS = 1e-8
DEN = LAMBDA_DRO * EPSILON + EPS
ACT_SCALE = float(LAMBDA_DRO / DEN)   # multiplies the psum before Exp

# AG1 block: normalized nu shard (bf16) + its column sums (f32)
AG1_BN = DC * 128 * ML                # 262144 bf16
AG1_SB = AG1_BN                       # f32[512] as 1024 bf16
AG1_BLK = AG1_BN + 2 * ML
# AG2 block: nu CE losses (f32[512]) + per-core max (f32[128])
AG2_NUL = 0
AG2_RMX = 2 * ML
AG2_BLK = 2 * ML + 2 * 128

_CACHE = {}


def _build_nc():
    import concourse.bass as bass
    import concourse.mybir as mybir
    import concourse.tile as tile
    from concourse import bacc, library_config

    dt = mybir.dt
    AF = mybir.ActivationFunctionType
    OP = mybir.AluOpType
    f32, bf16 = dt.float32, dt.bfloat16
    X = mybir.AxisListType.X

    nc = bacc.Bacc("TRN2", target_bir_lowering=False, debug=False, num_devices=R,
                   num_swdge_queues=1)

    # ---- parameters
    ptr_t = nc.declare_dram_parameter("ptr_t", [DC, 128, NL], f32, isOutput=False)
    nu_t = nc.declare_dram_parameter("nu_t", [DC, 128, ML], f32, isOutput=False)
    nu_lg = nc.declare_dram_parameter("nu_lg", [4, 128, K], f32, isOutput=False)
    w_in = nc.declare_dram_parameter("w", [WC, 128, D], f32, isOutput=False)
    ne_idx = nc.declare_dram_parameter("ne_idx", [128, M // 16], dt.int16, isOutput=False)
    pe_idx = nc.declare_dram_parameter("pe_idx", [128, NL // 16], dt.int16, isOutput=False)
    iota_f = nc.declare_dram_parameter("iota_f", [128, K], f32, isOutput=False)
    lbl_f = nc.declare_dram_parameter("lbl_f", [128, 4], f32, isOutput=False)
    out_p = nc.declare_dram_parameter("out", [4101], f32, isOutput=True)

    # ---- internal DRAM
    wn_dram = nc.dram_tensor("wn_dram", [WC * 128, D], bf16)
    ag1_in = nc.dram_tensor("ag1_in", [AG1_BLK], bf16)
    ag1_out = nc.dram_tensor("ag1_out", [R, AG1_BLK], bf16, addr_space="Shared")
    ag2_in = nc.dram_tensor("ag2_in", [AG2_BLK], bf16)
    ag2_out = nc.dram_tensor("ag2_out", [R, AG2_BLK], bf16, addr_space="Shared")
    ar_in = nc.dram_tensor("ar_in", [M + 264], f32)
    ar_out = nc.dram_tensor("ar_out", [M + 264], f32, addr_space="Shared")
    colv_dram = nc.dram_tensor("colv_dram", [2, M], bf16)
    fin_dram = nc.dram_tensor("fin_dram", [256], f32)

    rg = [list(range(R))]

    with tile.TileContext(nc, num_cores=R) as tc:
        with (
            tc.tile_pool(name="per", bufs=1) as per,
            tc.tile_pool(name="scr", bufs=3) as scr,
            tc.tile_pool(name="ez", bufs=2) as ezp,
            tc.tile_pool(name="mm", bufs=5, space="PSUM") as mmp,
            tc.tile_pool(name="aux", bufs=3, space="PSUM") as aux,
        ):
            nc.gpsimd.load_library(library_config.mlp)

            # constants
            ones_col_h = per.tile([128, 1], bf16, tag="ones_col_h")
            nc.vector.memset(ones_col_h[:], 1.0)
            ones_row_h = per.tile([1, 128], bf16, tag="ones_row_h")
            nc.vector.memset(ones_row_h[:], 1.0)
            cmat = per.tile([3, 128], bf16, tag="cmat")
            nc.vector.memset(cmat[:], 1.0)
            crhs = per.tile([3, M], bf16, tag="crhs")
            nc.vector.memset(crhs[:], 1.0)

            # ---------------- C. normalize this core's nu feature shard
            nu_ts = per.tile([128, DC, ML], f32, tag="nu_ts")
            nc.sync.dma_start(nu_ts[:], nu_t.rearrange("c p j -> p c j"))
            sqn = [scr.tile([128, D], bf16, tag="sqh", name=f"sqn{_k}")
                   for _k in range(DC)]
            for kc in range(DC):
                nc.scalar.activation(sqn[kc][:, 0:ML], nu_ts[:, kc], AF.Square)
            nn2_ps = aux.tile([1, ML], f32, tag="aux")
            for kc in range(DC):
                nc.tensor.matmul(nn2_ps[:], lhsT=ones_col_h[:], rhs=sqn[kc][:, 0:ML],
                                 start=(kc == 0), stop=(kc == DC - 1))
            nn = scr.tile([1, ML], f32, tag="nn")
            nc.scalar.activation(nn[:], nn2_ps[:], AF.Sqrt)
            rn = scr.tile([1, ML], bf16, tag="rn")
            with nc.allow_low_precision(reason="bf16 col-norm reciprocal"):
                nc.vector.reciprocal(rn[:], nn[:])
            rnB_ps = aux.tile([128, ML], f32, tag="aux")
            nc.tensor.matmul(rnB_ps[:], lhsT=ones_row_h[:], rhs=rn[:],
                             start=True, stop=True)
            bn_sh = per.tile([128, DC, ML], bf16, tag="bn_sh")
            sb4 = per.tile([128, DC], f32, tag="sb4")
            for kc in range(DC):
                nc.vector.tensor_tensor(bn_sh[:, kc], nu_ts[:, kc], rnB_ps[:], op=OP.mult)
                nc.vector.reduce_sum(sb4[:, kc:kc + 1], bn_sh[:, kc], axis=X)
            # AG1: bn shard + sb partials
            nc.sync.dma_start(
                ag1_in[0:AG1_BN].rearrange("(c p j) -> p c j", p=128, j=ML), bn_sh[:])
            nc.sync.dma_start(
                ag1_in[AG1_SB:AG1_SB + 2 * ML].bitcast(f32).rearrange("(c p) -> p c", p=128),
                sb4[:])
            nc.gpsimd.collective_compute(
                "AllGather", OP.bypass, replica_groups=rg,
                ins=[ag1_in[:].opt()], outs=[ag1_out[:].opt()],
            )

# CUTA
            # ---------------- A. normalize classifier rows -> wn (bf16) -> DRAM
            w_s = per.tile([128, WC, D], f32, tag="big16")
            nc.sync.dma_start(w_s[:], w_in.rearrange("c p d -> p c d"))
            w_n2 = per.tile([128, WC], f32, tag="w_n2")
            for wc in range(WC):
                sq = scr.tile([128, D], f32, tag="sq")
                nc.scalar.activation(sq[:], w_s[:, wc], AF.Square,
                                     accum_out=w_n2[:, wc:wc + 1])
            w_nrm = per.tile([128, WC], f32, tag="w_nrm")
            nc.scalar.activation(w_nrm[:], w_n2[:], AF.Sqrt)
            w_rn = per.tile([128, WC], f32, tag="w_rn")
            nc.vector.reciprocal(w_rn[:], w_nrm[:])
            wn_s = per.tile([128, WC, D], bf16, tag="wn_s")
            for wc in range(WC):
                nc.vector.tensor_scalar_mul(wn_s[:, wc], w_s[:, wc], w_rn[:, wc:wc + 1])
            nc.sync.dma_start(wn_dram.rearrange("(c p) d -> p c d", p=128), wn_s[:])

# CUTB
            # ---------------- B. gather label embeddings (transposed, bf16)
            ne_idx_s = per.tile([128, M // 16], dt.int16, tag="ne_idx_s")
            nc.sync.dma_start(ne_idx_s[:], ne_idx[:])
            pe_idx_s = per.tile([128, NL // 16], dt.int16, tag="pe_idx_s")
            nc.sync.dma_start(pe_idx_s[:], pe_idx[:])
            pr1 = scr.tile([1, M // 16], dt.int16, tag="pr1")
            nc.gpsimd.tensor_copy(pr1[:], ne_idx_s[0:1, :])
            pr2 = scr.tile([1, NL // 16], dt.int16, tag="pr2")
            nc.gpsimd.tensor_copy(pr2[:], pe_idx_s[0:1, :])
            pr3 = scr.tile([1, 16], bf16, tag="pr3")
            nc.gpsimd.dma_start(pr3[:], wn_dram[0:1, 0:16])
            ne_q = [per.tile([128, DC, 512], bf16, tag=f"ne_q{_q}", name=f"ne_q{_q}")
                    for _q in range(8)]
            for q in range(8):
                nc.gpsimd.dma_gather(out_ap=ne_q[q][:], in_ap=wn_dram[:, :],
                                     idxs_ap=ne_idx_s[:, q * 32:(q + 1) * 32],
                                     num_idxs=512, num_idxs_reg=512,
                                     elem_size=D, transpose=True,
                                     queue_num=0)
            pe_s = per.tile([128, DC, NL], bf16, tag="pe_s")
            nc.gpsimd.dma_gather(out_ap=pe_s[:], in_ap=wn_dram[:, :],
                                 idxs_ap=pe_idx_s[:], num_idxs=NL, num_idxs_reg=NL,
                                 elem_size=D, transpose=True, queue_num=0)
            snbh = per.tile([128, DC, 8], f32, tag="snbh")
            spa4 = per.tile([128, DC], f32, tag="spa4")
            for kc in range(DC):
                for q in range(8):
                    nc.vector.reduce_sum(snbh[:, kc, q:q + 1], ne_q[q][:, kc], axis=X)
                nc.vector.reduce_sum(spa4[:, kc:kc + 1], pe_s[:, kc], axis=X)
            snb4 = per.tile([128, DC], f32, tag="snb4")
            nc.vector.reduce_sum(snb4[:], snbh[:], axis=X)

# CUTD
            # ---------------- D. normalize ptr feature shard
            ptr_ts = per.tile([128, DC, NL], f32, tag="ptr_ts")
            nc.sync.dma_start(ptr_ts[:], ptr_t.rearrange("c p i -> p c i"))
            sqp = [scr.tile([128, D], bf16, tag="sqh", name=f"sqp{_k}")
                   for _k in range(DC)]
            for kc in range(DC):
                nc.scalar.activation(sqp[kc][:, 0:NL], ptr_ts[:, kc], AF.Square)
            pn2_ps = aux.tile([128, ML], f32, tag="aux", name="pn2_ps")[0:1, 0:NL]
            for kc in range(DC):
                nc.tensor.matmul(pn2_ps[:], lhsT=ones_col_h[:], rhs=sqp[kc][:, 0:NL],
                                 start=(kc == 0), stop=(kc == DC - 1))
            pn = scr.tile([1, NL], f32, tag="pn")
            nc.scalar.activation(pn[:], pn2_ps[:], AF.Sqrt)
            rp = scr.tile([1, NL], bf16, tag="rp")
            with nc.allow_low_precision(reason="bf16 col-norm reciprocal"):
                nc.vector.reciprocal(rp[:], pn[:])
            rpB_ps = aux.tile([128, ML], f32, tag="aux", name="rpB_ps")[:, 0:NL]
            nc.tensor.matmul(rpB_ps[:], lhsT=ones_row_h[:], rhs=rp[:],
                             start=True, stop=True)
            an_ts = per.tile([128, DC, NL], bf16, tag="an_ts")
            sa4 = per.tile([128, DC], f32, tag="sa4")
            for kc in range(DC):
                nc.vector.tensor_tensor(an_ts[:, kc], ptr_ts[:, kc], rpB_ps[:], op=OP.mult)
                nc.vector.reduce_sum(sa4[:, kc:kc + 1], an_ts[:, kc], axis=X)

# CUTE
            # ---------------- E. CE losses of the nu shard (pick via iota mask)
            lg_s = per.tile([128, 4, K], f32, tag="big16")
            nc.sync.dma_start(lg_s[:], nu_lg.rearrange("c p k -> p c k"))
            iota_s = per.tile([128, K], f32, tag="iota_s")
            nc.sync.dma_start(iota_s[:], iota_f[:])
            lbl_s = per.tile([128, 4], f32, tag="lbl_s")
            nc.sync.dma_start(lbl_s[:], lbl_f[:])
            sume = per.tile([128, 4], f32, tag="sume")
            pick = per.tile([128, 4], f32, tag="pick")
            for cc in range(4):
                ez = ezp.tile([128, K], f32, tag="ez")
                nc.scalar.activation(ez[:], lg_s[:, cc], AF.Exp,
                                     accum_out=sume[:, cc:cc + 1])
                msk = ezp.tile([128, K], f32, tag="msk")
                nc.vector.tensor_scalar(msk[:], iota_s[:], lbl_s[:, cc:cc + 1], None,
                                        OP.is_equal)
                nc.vector.tensor_tensor(msk[:], msk[:], lg_s[:, cc], op=OP.mult)
                nc.vector.reduce_sum(pick[:, cc:cc + 1], msk[:], axis=X)
            lse = per.tile([128, 4], f32, tag="lse")
            nc.scalar.activation(lse[:], sume[:], AF.Ln)
            nul4 = per.tile([128, 4], f32, tag="nul4")
            nc.vector.tensor_tensor(nul4[:], lse[:], pick[:], op=OP.subtract)
            rmax = per.tile([128, 1], f32, tag="rmax")
            nc.vector.reduce_max(rmax[:], nul4[:], axis=X)
# CUTE2
            # AG2: nul shard + rowmax
            nc.sync.dma_start(
                ag2_in[AG2_NUL:AG2_NUL + 2 * ML].bitcast(f32).rearrange("(c p) -> p c", p=128),
                nul4[:])
            nc.sync.dma_start(
                ag2_in[AG2_RMX:AG2_RMX + 2 * 128].bitcast(f32)[:, None], rmax[:])
            nc.gpsimd.collective_compute(
                "AllGather", OP.bypass, replica_groups=rg,
                ins=[ag2_in[:].opt()], outs=[ag2_out[:].opt()],
            )

# CUTG
            # ---------------- G. unpack gathered data
            bn_full = per.tile([128, DC, M], bf16, tag="bn_full")
            for r in range(R):
                nc.sync.dma_start(
                    bn_full[:, :, r * ML:(r + 1) * ML],
                    ag1_out[r, 0:AG1_BN].rearrange("(c p j) -> p c j", p=128, j=ML))
            sb8 = per.tile([128, DC, R], f32, tag="sb8")
            for r in range(R):
                nc.sync.dma_start(
                    sb8[:, :, r:r + 1],
                    ag1_out[r, AG1_SB:AG1_SB + 2 * ML].bitcast(f32).rearrange(
                        "(c p) -> p c", p=128)[:, :, None])
            sb_all = per.tile([128, DC], f32, tag="sb_all")
            nc.vector.reduce_sum(sb_all[:], sb8[:], axis=X)

            nul_rs = per.tile([128, M // 128], f32, tag="nul_rs")
            for r in range(R):
                nc.sync.dma_start(
                    nul_rs[16 * r:16 * (r + 1), :],
                    ag2_out[r, AG2_NUL:AG2_NUL + 2 * ML].bitcast(f32).rearrange(
                        "(p c) -> p c", p=16))
            mx_s = per.tile([1, 1024], f32, tag="mx_s")
            for r in range(R):
                nc.sync.dma_start(
                    mx_s[:, 128 * r:128 * (r + 1)],
                    ag2_out[r, AG2_RMX:AG2_RMX + 2 * 128].bitcast(f32)[None, :])

            numax = scr.tile([1, 1], f32, tag="numax")
            nc.vector.reduce_max(numax[:], mx_s[:], axis=X)
            c2 = scr.tile([1, 1], f32, tag="c2")
            nc.vector.tensor_scalar(c2[:], numax[:], -10.0, -2.0, OP.mult, OP.add)
            c2b = scr.tile([1, 128], bf16, tag="c2b")
            nc.vector.tensor_copy(c2b[:], c2[:].to_broadcast([1, 128]))
            nc.sync.dma_start(cmat[2:3, :], c2b[:])
            # hi/lo split of 10*nu_l into crhs rows 0/1
            t10 = scr.tile([128, 32], f32, tag="t10")
            nc.vector.tensor_scalar_mul(t10[:], nul_rs[:], 10.0)
            hi_rs = scr.tile([128, 32], bf16, tag="hi_rs")
            nc.vector.tensor_copy(hi_rs[:], t10[:])
            hif = scr.tile([128, 32], f32, tag="hif")
            nc.vector.tensor_copy(hif[:], hi_rs[:])
            lo_rs = scr.tile([128, 32], bf16, tag="lo_rs")
            nc.vector.tensor_tensor(lo_rs[:], t10[:], hif[:], op=OP.subtract)
            nc.sync.dma_start(colv_dram[0, :].rearrange("(p c) -> p c", p=128), hi_rs[:])
            nc.sync.dma_start(colv_dram[1, :].rearrange("(p c) -> p c", p=128), lo_rs[:])
            nc.sync.dma_start(crhs[0:2, :], colv_dram[:, :])

            # dot-product partials for the C means (ride the final AllReduce)
            dred2 = per.tile([128, 2], f32, tag="dred2")
            dts = scr.tile([128, DC], f32, tag="dts")
            for ci, (a, b) in enumerate(((sa4, sb_all), (spa4, snb4))):
                nc.vector.tensor_tensor(dts[:], a[:], b[:], op=OP.mult)
                nc.vector.reduce_sum(dred2[:, ci:ci + 1], dts[:], axis=X)
            nc.sync.dma_start(ar_in[M:M + 256].rearrange("(c p) -> p c", p=128), dred2[:])

# CUTH
            # ---------------- H. fused GEMMs + exp epilogue
            E_s = per.tile([128, 2, M], bf16, tag="E_s")
            racc = per.tile([128, 16], f32, tag="racc")
            tiles = [(it, jc) for it in range(2) for jc in range(8)]
            psums = {}

            def ne_half(t):
                it, jc = t
                pt = mmp.tile([128, 512], f32, tag="mmps", name=f"mmps_{t[0]}_{t[1]}")
                psums[t] = pt
                for kc in range(DC):
                    nc.tensor.matmul(pt[:], lhsT=pe_s[:, kc, it * 128:(it + 1) * 128],
                                     rhs=ne_q[jc][:, kc],
                                     start=(kc == 0), stop=False)

            def finish_tile(t):
                it, jc = t
                pt = psums.pop(t)
                for kc in range(DC):
                    nc.tensor.matmul(pt[:], lhsT=an_ts[:, kc, it * 128:(it + 1) * 128],
                                     rhs=bn_full[:, kc, jc * 512:(jc + 1) * 512],
                                     start=False, stop=False)
                nc.tensor.matmul(pt[:], lhsT=cmat[:],
                                 rhs=crhs[:, jc * 512:(jc + 1) * 512],
                                 start=False, stop=True)
                ti = it * 8 + jc
                nc.scalar.activation(E_s[:, it, jc * 512:(jc + 1) * 512], pt[:],
                                     AF.Exp, scale=ACT_SCALE,
                                     accum_out=racc[:, ti:ti + 1])

            for t in tiles[0:5]:
                ne_half(t)
            for i, t in enumerate(tiles):
                finish_tile(t)
                if i + 5 < len(tiles):
                    ne_half(tiles[i + 5])

            vb = per.tile([128, 2], bf16, tag="vb")
            for it in range(2):
                s1 = scr.tile([128, 1], f32, tag="s1")
                nc.vector.reduce_sum(s1[:], racc[:, it * 8:(it + 1) * 8], axis=X)
                s2 = scr.tile([128, 1], f32, tag="s2")
                nc.vector.tensor_scalar_mul(s2[:], s1[:], float(N))
                s3 = scr.tile([128, 1], bf16, tag="s3")
                with nc.allow_low_precision(reason="bf16 softmax scale"):
                    nc.vector.reciprocal(s3[:], s2[:])
                nc.vector.tensor_copy(vb[:, it:it + 1], s3[:])

            for jc in range(8):
                pps_t = aux.tile([128, ML], f32, tag="aux", name=f"pps{jc}")[0:1, :]
                for it in range(2):
                    nc.tensor.matmul(pps_t[:], lhsT=vb[:, it:it + 1],
                                     rhs=E_s[:, it, jc * 512:(jc + 1) * 512],
                                     start=(it == 0), stop=(it == 1))
                prow = scr.tile([1, 512], f32, tag="prow", name=f"prow{jc}")
                nc.scalar.copy(prow[:], pps_t[:])
                nc.sync.dma_start(ar_in[jc * 512:(jc + 1) * 512][None, :], prow[:])

# CUTI
            # ---------------- I. AllReduce (probs + dot partials)
            nc.gpsimd.collective_compute(
                "AllReduce", OP.add, replica_groups=rg,
                ins=[ar_in[:].opt()], outs=[ar_out[:].opt()],
            )

# CUTJ
            # ---------------- J. final assembly (identical on all cores)
            pr_s = per.tile([128, 32], f32, tag="pr_s")
            nc.sync.dma_start(pr_s[:], ar_out[0:M].rearrange("(p c) -> p c", p=128))
            nc.sync.dma_start(out_p[0:M].rearrange("(p c) -> p c", p=128), pr_s[:])
            dd = scr.tile([128, 32], f32, tag="dd")
            ddr = scr.tile([128, 1], f32, tag="ddr")
            nc.vector.tensor_tensor(dd[:], pr_s[:], nul_rs[:], op=OP.mult)
            nc.vector.reduce_sum(ddr[:], dd[:], axis=X)
            nc.sync.dma_start(fin_dram[0:128][:, None], ddr[:])
            drow = scr.tile([1, 128], f32, tag="drow")
            nc.sync.dma_start(drow[:], fin_dram[0:128][None, :])
            dro_s = scr.tile([1, 1], f32, tag="dro_s")
            nc.vector.reduce_sum(dro_s[:], drow[:], axis=X)
            nc.sync.dma_start(out_p[4096:4097][None, :], dro_s[:])
            nc.sync.dma_start(out_p[4097:4098][None, :], dro_s[:])

            dxyrow = scr.tile([1, 2, 128], f32, tag="dxyrow")
            nc.sync.dma_start(
                dxyrow[:],
                ar_out[M:M + 256].rearrange("(c p) -> p c", p=128).rearrange(
                    "p c -> c p")[None])
            dxy = scr.tile([1, 2], f32, tag="dxy")
            nc.vector.reduce_sum(dxy[:], dxyrow[:], axis=X)
            dsum = scr.tile([1, 1], f32, tag="dsum")
            nc.vector.reduce_sum(dsum[:], dxy[:], axis=X)
            mc = scr.tile([1, 1], f32, tag="mc")
            nc.vector.tensor_scalar(mc[:], dsum[:], -1.0 / (N * M), 2.0, OP.mult, OP.add)
            nc.sync.dma_start(out_p[4098:4099][None, :], mc[:])
            mcx = scr.tile([1, 1], f32, tag="mcx")
            nc.vector.tensor_scalar(mcx[:], dxy[:, 0:1], -1.0 / (N * M), 1.0, OP.mult, OP.add)
            nc.sync.dma_start(out_p[4099:4100][None, :], mcx[:])
            mcy = scr.tile([1, 1], f32, tag="mcy")
            nc.vector.tensor_scalar(mcy[:], dxy[:, 1:2], -1.0 / (N * M), 1.0, OP.mult, OP.add)
            nc.sync.dma_start(out_p[4100:4101][None, :], mcy[:])

    nc.compile()
    return nc


def _wrap16(idx):
    n = idx.shape[0]
    a = np.ascontiguousarray(idx.reshape(n // 16, 16).T.astype(np.int16))
    return np.ascontiguousarray(np.tile(a, (8, 1)))  # [128, n/16]


def make_in_maps(ptr_features, ptr_labels, nu_features, nu_logits, nu_labels,
                 classifier_weights):
    w_pad = np.concatenate(
        [np.asarray(classifier_weights, np.float32),
         np.ones((WC * 128 - K, D), np.float32)]).reshape(WC, 128, D)
    ne = _wrap16(np.asarray(nu_labels).astype(np.int64))
    iota = np.broadcast_to(np.arange(K, dtype=np.float32), (128, K)).copy()
    in_maps = []
    for c in range(R):
        p_sh = np.asarray(ptr_features[c * NL:(c + 1) * NL], np.float32)
        nu_sh = np.asarray(nu_features[c * ML:(c + 1) * ML], np.float32)
        lg_sh = np.ascontiguousarray(
            np.asarray(nu_logits[c * ML:(c + 1) * ML], np.float32)).reshape(4, 128, K)
        lbl = np.asarray(nu_labels[c * ML:(c + 1) * ML]).astype(np.int64)
        lblf = np.ascontiguousarray(lbl.reshape(4, 128).T).astype(np.float32)
        in_maps.append({
            "ptr_t": np.ascontiguousarray(p_sh.T).reshape(DC, 128, NL),
            "nu_t": np.ascontiguousarray(nu_sh.T).reshape(DC, 128, ML),
            "nu_lg": lg_sh,
            "w": np.ascontiguousarray(w_pad),
            "ne_idx": ne,
            "pe_idx": _wrap16(np.asarray(ptr_labels[c * NL:(c + 1) * NL]).astype(np.int64)),
            "iota_f": iota,
            "lbl_f": lblf,
        })
    return in_maps


def kernel(ptr_features, ptr_logits, ptr_labels, nu_features, nu_logits,
           nu_labels, classifier_weights):
    from concourse.bass_utils import run_bass_kernel_spmd

    if "nc" not in _CACHE:
        _CACHE["nc"] = _build_nc()
    nc = _CACHE["nc"]
    in_maps = make_in_maps(ptr_features, ptr_labels, nu_features, nu_logits,
                           nu_labels, classifier_weights)
    res = run_bass_kernel_spmd(nc, in_maps, core_ids=list(range(R)))
    o = np.asarray(res.results[0]["out"], np.float32)
    probs = o[0:M].copy()
    return (np.float32(o[4096]), np.float32(o[4097]), probs,
            np.float32(o[4098]), np.float32(o[4099]), np.float32(o[4100]))


# revision 27
# speedup vs baseline: 1.0319x; 1.0319x over previous
"""AdaDRO loss kernel for 8 TRN2 NeuronCores (Bass/Tile, SPMD).

Sharding: N=2048 ptr rows split across 8 cores (256 each); per-nu work
(feature row-normalize, CE losses) sharded by M/8=512 and all-gathered.
classifier_weights are row-normalized to bf16 once per core; label
embeddings are fetched with dma_gather(transpose=True) directly into the
[d%128, d//128, j] matmul layout.  Both cosine GEMMs plus a bf16 rank-3
term (hi/lo split of the CE-loss column + constant) accumulate into one
PSUM tile per output block, so a single Exp activation computes the
softmax numerator with row sums accumulated for free.  Collectives: an
early AllGather for the normalized nu shard, a tiny AllGather for the CE
losses/max, and a final 17KB AllReduce for the worst-case probs.
"""
import numpy as np

N, M, D, K = 2048, 4096, 512, 1000
R = 8
NL = N // R        # 256 ptr rows per core
ML = M // R        # 512 nu rows per core
DC = D // 128      # 4 d-chunks
WC = 1024 // 128   # 8 w row chunks (K padded 1000 -> 1024)
LAMBDA_DRO = 0.1
EPSILON = 0.5
EPS = 1e-8
DEN = LAMBDA_DRO * EPSILON + EPS
ACT_SCALE = float(LAMBDA_DRO / DEN)   # multiplies the psum before Exp

# AG1 block: normalized nu shard (bf16) + its column sums (f32)
AG1_BN = DC * 128 * ML                # 262144 bf16
AG1_SB = AG1_BN                       # f32[512] as 1024 bf16
AG1_BLK = AG1_BN + 2 * ML
# AG2 block: nu CE losses (f32[512]) + per-core max (f32[128])
AG2_NUL = 0
AG2_RMX = 2 * ML
AG2_BLK = 2 * ML + 2 * 128

_CACHE = {}


def _build_nc():
    import concourse.bass as bass
    import concourse.mybir as mybir
    import concourse.tile as tile
    from concourse import bacc, library_config

    dt = mybir.dt
    AF = mybir.ActivationFunctionType
    OP = mybir.AluOpType
    f32, bf16 = dt.float32, dt.bfloat16
    X = mybir.AxisListType.X

    nc = bacc.Bacc("TRN2", target_bir_lowering=False, debug=False, num_devices=R,
                   num_swdge_queues=1)

    # ---- parameters
    ptr_t = nc.declare_dram_parameter("ptr_t", [DC, 128, NL], f32, isOutput=False)
    nu_t = nc.declare_dram_parameter("nu_t", [DC, 128, ML], f32, isOutput=False)
    nu_lg = nc.declare_dram_parameter("nu_lg", [4, 128, K], f32, isOutput=False)
    w_in = nc.declare_dram_parameter("w", [WC, 128, D], f32, isOutput=False)
    ne_idx = nc.declare_dram_parameter("ne_idx", [128, M // 16], dt.int16, isOutput=False)
    pe_idx = nc.declare_dram_parameter("pe_idx", [128, NL // 16], dt.int16, isOutput=False)
    iota_f = nc.declare_dram_parameter("iota_f", [128, K], f32, isOutput=False)
    lbl_f = nc.declare_dram_parameter("lbl_f", [128, 4], f32, isOutput=False)
    out_p = nc.declare_dram_parameter("out", [4101], f32, isOutput=True)

    # ---- internal DRAM
    wn_dram = nc.dram_tensor("wn_dram", [WC * 128, D], bf16)
    ag1_in = nc.dram_tensor("ag1_in", [AG1_BLK], bf16)
    ag1_out = nc.dram_tensor("ag1_out", [R, AG1_BLK], bf16, addr_space="Shared")
    ag2_in = nc.dram_tensor("ag2_in", [AG2_BLK], bf16)
    ag2_out = nc.dram_tensor("ag2_out", [R, AG2_BLK], bf16, addr_space="Shared")
    ar_in = nc.dram_tensor("ar_in", [M + 264], f32)
    ar_out = nc.dram_tensor("ar_out", [M + 264], f32, addr_space="Shared")
    fin_dram = nc.dram_tensor("fin_dram", [256], f32)

    rg = [list(range(R))]

    with tile.TileContext(nc, num_cores=R) as tc:
        with (
            tc.tile_pool(name="per", bufs=1) as per,
            tc.tile_pool(name="scr", bufs=1) as scr,
            tc.tile_pool(name="sqp_", bufs=2) as sqpool,
            tc.tile_pool(name="prowp", bufs=2) as prowp,
            tc.tile_pool(name="ez", bufs=1) as ezp,
            tc.tile_pool(name="mskp", bufs=2) as mskp,
            tc.tile_pool(name="mm", bufs=5, space="PSUM") as mmp,
            tc.tile_pool(name="aux", bufs=3, space="PSUM") as aux,
        ):
            nc.gpsimd.load_library(library_config.mlp)

            # constants
            ones_col_h = per.tile([128, 1], bf16, tag="ones_col_h")
            nc.vector.memset(ones_col_h[:], 1.0)
            ones_row_h = per.tile([1, 128], bf16, tag="ones_row_h")
            nc.vector.memset(ones_row_h[:], 1.0)
            cmat = per.tile([3, 128], bf16, tag="cmat")
            nc.vector.memset(cmat[:], 1.0)
            crhs = per.tile([3, M], bf16, tag="crhs")
            nc.vector.memset(crhs[:], 1.0)

# CUTA
            # ---------------- A. normalize classifier rows -> wn (bf16) -> DRAM
            w_s = per.tile([128, WC, D], f32, tag="big16")
            nc.sync.dma_start(w_s[:], w_in.rearrange("c p d -> p c d"))
            w_n2 = per.tile([128, WC], f32, tag="w_n2")
            for wc in range(WC):
                sq = sqpool.tile([128, D], f32, tag="sq")
                nc.scalar.activation(sq[:], w_s[:, wc], AF.Square,
                                     accum_out=w_n2[:, wc:wc + 1])
            w_nrm = per.tile([128, WC], f32, tag="w_nrm")
            nc.scalar.activation(w_nrm[:], w_n2[:], AF.Sqrt)
            w_rn = per.tile([128, WC], f32, tag="w_rn")
            nc.vector.reciprocal(w_rn[:], w_nrm[:])
            wn_s = per.tile([128, 2, M], bf16, tag="bhalf", name="wn_s")[:, 0, :].rearrange("p (c d) -> p c d", d=D)
            for wc in range(WC):
                nc.vector.tensor_scalar_mul(wn_s[:, wc], w_s[:, wc], w_rn[:, wc:wc + 1])
            nc.sync.dma_start(wn_dram.rearrange("(c p) d -> p c d", p=128), wn_s[:])

            # ---------------- C. normalize this core's nu feature shard
            nu_ts = per.tile([128, DC, M], bf16, tag="big32", name="nu_ts")[:, :, 0:2 * ML].bitcast(f32)
            nc.sync.dma_start(nu_ts[:], nu_t.rearrange("c p j -> p c j"))
            sqn = [sqpool.tile([128, D], bf16, tag="sqh", name=f"sqn{_k}")
                   for _k in range(DC)]
            for kc in range(DC):
                nc.scalar.activation(sqn[kc][:, 0:ML], nu_ts[:, kc], AF.Square)
            nn2_ps = aux.tile([1, ML], f32, tag="aux")
            for kc in range(DC):
                nc.tensor.matmul(nn2_ps[:], lhsT=ones_col_h[:], rhs=sqn[kc][:, 0:ML],
                                 start=(kc == 0), stop=(kc == DC - 1))
            nn = scr.tile([1, ML], f32, tag="nn")
            nc.scalar.activation(nn[:], nn2_ps[:], AF.Sqrt)
            rn = scr.tile([1, ML], bf16, tag="rn")
            with nc.allow_low_precision(reason="bf16 col-norm reciprocal"):
                nc.vector.reciprocal(rn[:], nn[:])
            rnB_ps = aux.tile([128, ML], f32, tag="aux")
            nc.tensor.matmul(rnB_ps[:], lhsT=ones_row_h[:], rhs=rn[:],
                             start=True, stop=True)
            bn_sh = per.tile([128, DC, ML], bf16, tag="bn_sh")
            sb4 = per.tile([128, DC], f32, tag="sb4")
            for kc in range(DC):
                nc.vector.tensor_tensor(bn_sh[:, kc], nu_ts[:, kc], rnB_ps[:], op=OP.mult)
                nc.vector.reduce_sum(sb4[:, kc:kc + 1], bn_sh[:, kc], axis=X)
            # AG1: bn shard + sb partials
            nc.sync.dma_start(
                ag1_in[0:AG1_BN].rearrange("(c p j) -> p c j", p=128, j=ML), bn_sh[:])
            nc.sync.dma_start(
                ag1_in[AG1_SB:AG1_SB + 2 * ML].bitcast(f32).rearrange("(c p) -> p c", p=128),
                sb4[:])
            nc.gpsimd.collective_compute(
                "AllGather", OP.bypass, replica_groups=rg,
                ins=[ag1_in[:].opt()], outs=[ag1_out[:].opt()],
            )

# CUTB
            # ---------------- B. gather label embeddings (transposed, bf16)
            ne_idx_s = per.tile([128, M // 16], dt.int16, tag="ne_idx_s")
            nc.sync.dma_start(ne_idx_s[:], ne_idx[:])
            pe_idx_s = per.tile([128, NL // 16], dt.int16, tag="pe_idx_s")
            nc.sync.dma_start(pe_idx_s[:], pe_idx[:])
            pr1 = scr.tile([1, M // 16], dt.int16, tag="pr1")
            nc.gpsimd.tensor_copy(pr1[:], ne_idx_s[0:1, :])
            pr2 = scr.tile([1, NL // 16], dt.int16, tag="pr2")
            nc.gpsimd.tensor_copy(pr2[:], pe_idx_s[0:1, :])
            pr3 = scr.tile([1, 16], bf16, tag="pr3")
            nc.gpsimd.dma_start(pr3[:], wn_dram[0:1, 0:16])
            ne_q = [per.tile([128, DC, 512], bf16, tag=f"ne_q{_q}", name=f"ne_q{_q}")
                    for _q in range(8)]
            for q in range(8):
                nc.gpsimd.dma_gather(out_ap=ne_q[q][:], in_ap=wn_dram[:, :],
                                     idxs_ap=ne_idx_s[:, q * 32:(q + 1) * 32],
                                     num_idxs=512, num_idxs_reg=512,
                                     elem_size=D, transpose=True,
                                     queue_num=0)
            pe_s = per.tile([128, DC, NL], bf16, tag="pe_s")
            nc.gpsimd.dma_gather(out_ap=pe_s[:], in_ap=wn_dram[:, :],
                                 idxs_ap=pe_idx_s[:], num_idxs=NL, num_idxs_reg=NL,
                                 elem_size=D, transpose=True, queue_num=0)
            snbh = per.tile([128, DC, 8], f32, tag="snbh")
            spa4 = per.tile([128, DC], f32, tag="spa4")
            for kc in range(DC):
                for q in range(8):
                    nc.vector.reduce_sum(snbh[:, kc, q:q + 1], ne_q[q][:, kc], axis=X)
                nc.vector.reduce_sum(spa4[:, kc:kc + 1], pe_s[:, kc], axis=X)
            snb4 = per.tile([128, DC], f32, tag="snb4")
            nc.vector.reduce_sum(snb4[:], snbh[:], axis=X)

# CUTD
            # ---------------- D. normalize ptr feature shard
            ptr_ts = per.tile([128, DC, NL], f32, tag="ptr_ts")
            nc.sync.dma_start(ptr_ts[:], ptr_t.rearrange("c p i -> p c i"))
            sqp = [sqpool.tile([128, D], bf16, tag="sqh", name=f"sqp{_k}")
                   for _k in range(DC)]
            for kc in range(DC):
                nc.scalar.activation(sqp[kc][:, 0:NL], ptr_ts[:, kc], AF.Square)
            pn2_ps = aux.tile([128, ML], f32, tag="aux", name="pn2_ps")[0:1, 0:NL]
            for kc in range(DC):
                nc.tensor.matmul(pn2_ps[:], lhsT=ones_col_h[:], rhs=sqp[kc][:, 0:NL],
                                 start=(kc == 0), stop=(kc == DC - 1))
            pn = scr.tile([1, NL], f32, tag="pn")
            nc.scalar.activation(pn[:], pn2_ps[:], AF.Sqrt)
            rp = scr.tile([1, NL], bf16, tag="rp")
            with nc.allow_low_precision(reason="bf16 col-norm reciprocal"):
                nc.vector.reciprocal(rp[:], pn[:])
            rpB_ps = aux.tile([128, ML], f32, tag="aux", name="rpB_ps")[:, 0:NL]
            nc.tensor.matmul(rpB_ps[:], lhsT=ones_row_h[:], rhs=rp[:],
                             start=True, stop=True)
            an_ts = per.tile([128, DC, NL], bf16, tag="an_ts")
            sa4 = per.tile([128, DC], f32, tag="sa4")
            for kc in range(DC):
                nc.vector.tensor_tensor(an_ts[:, kc], ptr_ts[:, kc], rpB_ps[:], op=OP.mult)
                nc.vector.reduce_sum(sa4[:, kc:kc + 1], an_ts[:, kc], axis=X)

# CUTE
            # ---------------- E. CE losses of the nu shard (pick via iota mask)
            lg_s = per.tile([128, 4, K], f32, tag="big16")
            nc.sync.dma_start(lg_s[:], nu_lg.rearrange("c p k -> p c k"))
            iota_s = per.tile([128, K], f32, tag="iota_s")
            nc.sync.dma_start(iota_s[:], iota_f[:])
            lbl_s = per.tile([128, 4], f32, tag="lbl_s")
            nc.sync.dma_start(lbl_s[:], lbl_f[:])
            sume = per.tile([128, 4], f32, tag="sume")
            pick = per.tile([128, 4], f32, tag="pick")
            for cc in range(4):
                ez = ezp.tile([128, K], f32, tag="ez")
                nc.scalar.activation(ez[:], lg_s[:, cc], AF.Exp,
                                     accum_out=sume[:, cc:cc + 1])
                msk = mskp.tile([128, K], f32, tag="msk")
                nc.vector.tensor_scalar(msk[:], iota_s[:], lbl_s[:, cc:cc + 1], None,
                                        OP.is_equal)
                nc.vector.tensor_tensor(msk[:], msk[:], lg_s[:, cc], op=OP.mult)
                nc.vector.reduce_sum(pick[:, cc:cc + 1], msk[:], axis=X)
            lse = per.tile([128, 4], f32, tag="lse")
            nc.scalar.activation(lse[:], sume[:], AF.Ln)
            nul4 = per.tile([128, 4], f32, tag="nul4")
            nc.vector.tensor_tensor(nul4[:], lse[:], pick[:], op=OP.subtract)
            rmax = per.tile([128, 1], f32, tag="rmax")
            nc.vector.reduce_max(rmax[:], nul4[:], axis=X)
# CUTE2
            # AG2: nul shard + rowmax
            nc.sync.dma_start(
                ag2_in[AG2_NUL:AG2_NUL + 2 * ML].bitcast(f32).rearrange("(c p) -> p c", p=128),
                nul4[:])
            nc.sync.dma_start(
                ag2_in[AG2_RMX:AG2_RMX + 2 * 128].bitcast(f32)[:, None], rmax[:])
            nc.gpsimd.collective_compute(
                "AllGather", OP.bypass, replica_groups=rg,
                ins=[ag2_in[:].opt()], outs=[ag2_out[:].opt()],
            )

# CUTG
            # ---------------- G. unpack gathered data
            bn_full = per.tile([128, DC, M], bf16, tag="big32", name="bn_full")
            for r in range(R):
                nc.sync.dma_start(
                    bn_full[:, :, r * ML:(r + 1) * ML],
                    ag1_out[r, 0:AG1_BN].rearrange("(c p j) -> p c j", p=128, j=ML))
            sb8 = per.tile([128, DC, R], f32, tag="sb8")
            for r in range(R):
                nc.sync.dma_start(
                    sb8[:, :, r:r + 1],
                    ag1_out[r, AG1_SB:AG1_SB + 2 * ML].bitcast(f32).rearrange(
                        "(c p) -> p c", p=128)[:, :, None])
            sb_all = per.tile([128, DC], f32, tag="sb_all")
            nc.vector.reduce_sum(sb_all[:], sb8[:], axis=X)

            nul_rs = per.tile([128, M // 128], f32, tag="nul_rs")
            for r in range(R):
                nc.sync.dma_start(
                    nul_rs[16 * r:16 * (r + 1), :],
                    ag2_out[r, AG2_NUL:AG2_NUL + 2 * ML].bitcast(f32).rearrange(
                        "(p c) -> p c", p=16))
            mx_s = per.tile([1, 1024], f32, tag="mx_s")
            for r in range(R):
                nc.sync.dma_start(
                    mx_s[:, 128 * r:128 * (r + 1)],
                    ag2_out[r, AG2_RMX:AG2_RMX + 2 * 128].bitcast(f32)[None, :])

            numax = scr.tile([1, 1], f32, tag="numax")
            nc.vector.reduce_max(numax[:], mx_s[:], axis=X)
            c2 = scr.tile([1, 1], f32, tag="c2")
            nc.vector.tensor_scalar(c2[:], numax[:], -10.0, -2.0, OP.mult, OP.add)
            c2b = scr.tile([1, 128], bf16, tag="c2b")
            nc.vector.tensor_copy(c2b[:], c2[:].to_broadcast([1, 128]))
            nc.sync.dma_start(cmat[2:3, :], c2b[:])
            # hi/lo split of 10*nu_l into crhs rows 0/1
            t10 = scr.tile([128, 32], f32, tag="t10")
            nc.vector.tensor_scalar_mul(t10[:], nul_rs[:], 10.0)
            hi_rs = scr.tile([128, 32], bf16, tag="hi_rs")
            nc.vector.tensor_copy(hi_rs[:], t10[:])
            hif = scr.tile([128, 32], f32, tag="hif")
            nc.vector.tensor_copy(hif[:], hi_rs[:])
            lo_rs = scr.tile([128, 32], bf16, tag="lo_rs")
            nc.vector.tensor_tensor(lo_rs[:], t10[:], hif[:], op=OP.subtract)
            nc.sync.dma_start(crhs[0:1, :], hi_rs[:])
            nc.sync.dma_start(crhs[1:2, :], lo_rs[:])

            # dot-product partials for the C means (ride the final AllReduce)
            dred2 = per.tile([128, 2], f32, tag="dred2")
            dts = scr.tile([128, DC], f32, tag="dts")
            for ci, (a, b) in enumerate(((sa4, sb_all), (spa4, snb4))):
                nc.vector.tensor_tensor(dts[:], a[:], b[:], op=OP.mult)
                nc.vector.reduce_sum(dred2[:, ci:ci + 1], dts[:], axis=X)
            nc.sync.dma_start(ar_in[M:M + 256].rearrange("(c p) -> p c", p=128), dred2[:])

# CUTH
            # ---------------- H. fused GEMMs + exp epilogue
            E_s = per.tile([128, 2, M], bf16, tag="bhalf", name="E_s")
            racc = per.tile([128, 16], f32, tag="racc")
            tiles = [(it, jc) for it in range(2) for jc in range(8)]
            psums = {}

            def ne_half(t):
                it, jc = t
                pt = mmp.tile([128, 512], f32, tag="mmps", name=f"mmps_{t[0]}_{t[1]}")
                psums[t] = pt
                for kc in range(DC):
                    nc.tensor.matmul(pt[:], lhsT=pe_s[:, kc, it * 128:(it + 1) * 128],
                                     rhs=ne_q[jc][:, kc],
                                     start=(kc == 0), stop=False)

            def finish_tile(t):
                it, jc = t
                pt = psums.pop(t)
                for kc in range(DC):
                    nc.tensor.matmul(pt[:], lhsT=an_ts[:, kc, it * 128:(it + 1) * 128],
                                     rhs=bn_full[:, kc, jc * 512:(jc + 1) * 512],
                                     start=False, stop=False)
                nc.tensor.matmul(pt[:], lhsT=cmat[:],
                                 rhs=crhs[:, jc * 512:(jc + 1) * 512],
                                 start=False, stop=True)
                ti = it * 8 + jc
                nc.scalar.activation(E_s[:, it, jc * 512:(jc + 1) * 512], pt[:],
                                     AF.Exp, scale=ACT_SCALE,
                                     accum_out=racc[:, ti:ti + 1])

            for t in tiles[0:5]:
                ne_half(t)
            for i, t in enumerate(tiles):
                finish_tile(t)
                if i + 5 < len(tiles):
                    ne_half(tiles[i + 5])

            vb = per.tile([128, 2], bf16, tag="vb")
            for it in range(2):
                s1 = scr.tile([128, 1], f32, tag="s1")
                nc.vector.reduce_sum(s1[:], racc[:, it * 8:(it + 1) * 8], axis=X)
                s2 = scr.tile([128, 1], f32, tag="s2")
                nc.vector.tensor_scalar_mul(s2[:], s1[:], float(N))
                s3 = scr.tile([128, 1], bf16, tag="s3")
                with nc.allow_low_precision(reason="bf16 softmax scale"):
                    nc.vector.reciprocal(s3[:], s2[:])
                nc.vector.tensor_copy(vb[:, it:it + 1], s3[:])

            for jc in range(8):
                pps_t = aux.tile([128, ML], f32, tag="aux", name=f"pps{jc}")[0:1, :]
                for it in range(2):
                    nc.tensor.matmul(pps_t[:], lhsT=vb[:, it:it + 1],
                                     rhs=E_s[:, it, jc * 512:(jc + 1) * 512],
                                     start=(it == 0), stop=(it == 1))
                prow = prowp.tile([1, 512], f32, tag="prow", name=f"prow{jc}")
                nc.scalar.copy(prow[:], pps_t[:])
                nc.sync.dma_start(ar_in[jc * 512:(jc + 1) * 512][None, :], prow[:])

# CUTI
            # ---------------- I. AllReduce (probs + dot partials)
            nc.gpsimd.collective_compute(
                "AllReduce", OP.add, replica_groups=rg,
                ins=[ar_in[:].opt()], outs=[ar_out[:].opt()],
            )

# CUTJ
            # ---------------- J. final assembly (identical on all cores)
            nc.sync.dma_start(out_p[0:M], ar_out[0:M])
            pr_s = per.tile([128, 32], f32, tag="pr_s")
            nc.sync.dma_start(pr_s[:], ar_out[0:M].rearrange("(p c) -> p c", p=128))
            dd = scr.tile([128, 32], f32, tag="dd")
            ddr = scr.tile([128, 1], f32, tag="ddr")
            nc.vector.tensor_tensor(dd[:], pr_s[:], nul_rs[:], op=OP.mult)
            nc.vector.reduce_sum(ddr[:], dd[:], axis=X)
            nc.sync.dma_start(fin_dram[0:128][:, None], ddr[:])
            drow = scr.tile([1, 128], f32, tag="drow")
            nc.sync.dma_start(drow[:], fin_dram[0:128][None, :])
            dro_s = scr.tile([1, 1], f32, tag="dro_s")
            nc.vector.reduce_sum(dro_s[:], drow[:], axis=X)
            nc.sync.dma_start(out_p[4096:4097][None, :], dro_s[:])
            nc.sync.dma_start(out_p[4097:4098][None, :], dro_s[:])

            dxyrow = scr.tile([1, 2, 128], f32, tag="dxyrow")
            nc.sync.dma_start(
                dxyrow[:],
                ar_out[M:M + 256].rearrange("(c p) -> p c", p=128).rearrange(
                    "p c -> c p")[None])
            dxy = scr.tile([1, 2], f32, tag="dxy")
            nc.vector.reduce_sum(dxy[:], dxyrow[:], axis=X)
            dsum = scr.tile([1, 1], f32, tag="dsum")
            nc.vector.reduce_sum(dsum[:], dxy[:], axis=X)
            mc = scr.tile([1, 1], f32, tag="mc")
            nc.vector.tensor_scalar(mc[:], dsum[:], -1.0 / (N * M), 2.0, OP.mult, OP.add)
            nc.sync.dma_start(out_p[4098:4099][None, :], mc[:])
            mcx = scr.tile([1, 1], f32, tag="mcx")
            nc.vector.tensor_scalar(mcx[:], dxy[:, 0:1], -1.0 / (N * M), 1.0, OP.mult, OP.add)
            nc.sync.dma_start(out_p[4099:4100][None, :], mcx[:])
            mcy = scr.tile([1, 1], f32, tag="mcy")
            nc.vector.tensor_scalar(mcy[:], dxy[:, 1:2], -1.0 / (N * M), 1.0, OP.mult, OP.add)
            nc.sync.dma_start(out_p[4100:4101][None, :], mcy[:])

    nc.compile()
    return nc


def _wrap16(idx):
    n = idx.shape[0]
    a = np.ascontiguousarray(idx.reshape(n // 16, 16).T.astype(np.int16))
    return np.ascontiguousarray(np.tile(a, (8, 1)))  # [128, n/16]


def make_in_maps(ptr_features, ptr_labels, nu_features, nu_logits, nu_labels,
                 classifier_weights):
    w_pad = np.concatenate(
        [np.asarray(classifier_weights, np.float32),
         np.ones((WC * 128 - K, D), np.float32)]).reshape(WC, 128, D)
    ne = _wrap16(np.asarray(nu_labels).astype(np.int64))
    iota = np.broadcast_to(np.arange(K, dtype=np.float32), (128, K)).copy()
    in_maps = []
    for c in range(R):
        p_sh = np.asarray(ptr_features[c * NL:(c + 1) * NL], np.float32)
        nu_sh = np.asarray(nu_features[c * ML:(c + 1) * ML], np.float32)
        lg_sh = np.ascontiguousarray(
            np.asarray(nu_logits[c * ML:(c + 1) * ML], np.float32)).reshape(4, 128, K)
        lbl = np.asarray(nu_labels[c * ML:(c + 1) * ML]).astype(np.int64)
        lblf = np.ascontiguousarray(lbl.reshape(4, 128).T).astype(np.float32)
        in_maps.append({
            "ptr_t": np.ascontiguousarray(p_sh.T).reshape(DC, 128, NL),
            "nu_t": np.ascontiguousarray(nu_sh.T).reshape(DC, 128, ML),
            "nu_lg": lg_sh,
            "w": np.ascontiguousarray(w_pad),
            "ne_idx": ne,
            "pe_idx": _wrap16(np.asarray(ptr_labels[c * NL:(c + 1) * NL]).astype(np.int64)),
            "iota_f": iota,
            "lbl_f": lblf,
        })
    return in_maps


def kernel(ptr_features, ptr_logits, ptr_labels, nu_features, nu_logits,
           nu_labels, classifier_weights):
    from concourse.bass_utils import run_bass_kernel_spmd

    if "nc" not in _CACHE:
        _CACHE["nc"] = _build_nc()
    nc = _CACHE["nc"]
    in_maps = make_in_maps(ptr_features, ptr_labels, nu_features, nu_logits,
                           nu_labels, classifier_weights)
    res = run_bass_kernel_spmd(nc, in_maps, core_ids=list(range(R)))
    o = np.asarray(res.results[0]["out"], np.float32)
    probs = o[0:M].copy()
    return (np.float32(o[4096]), np.float32(o[4097]), probs,
            np.float32(o[4098]), np.float32(o[4099]), np.float32(o[4100]))


# revision 28
# speedup vs baseline: 1.0721x; 1.0389x over previous
"""AdaDRO loss kernel for 8 TRN2 NeuronCores (Bass/Tile, SPMD).

Sharding: N=2048 ptr rows split across 8 cores (256 each); per-nu work
(feature row-normalize, CE losses) sharded by M/8=512 and all-gathered.
classifier_weights are row-normalized to bf16 once per core; label
embeddings are fetched with dma_gather(transpose=True) directly into the
[d%128, d//128, j] matmul layout.  Both cosine GEMMs plus a bf16 rank-3
term (hi/lo split of the CE-loss column + constant) accumulate into one
PSUM tile per output block, so a single Exp activation computes the
softmax numerator with row sums accumulated for free.  Collectives: an
early AllGather for the normalized nu shard, a tiny AllGather for the CE
losses/max, and a final 17KB AllReduce for the worst-case probs.
"""
import numpy as np

N, M, D, K = 2048, 4096, 512, 1000
R = 8
NL = N // R        # 256 ptr rows per core
ML = M // R        # 512 nu rows per core
DC = D // 128      # 4 d-chunks
WC = 1024 // 128   # 8 w row chunks (K padded 1000 -> 1024)
LAMBDA_DRO = 0.1
EPSILON = 0.5
EPS = 1e-8
DEN = LAMBDA_DRO * EPSILON + EPS
ACT_SCALE = float(LAMBDA_DRO / DEN)   # multiplies the psum before Exp

# AG block: normalized nu shard (bf16) + col sums + CE losses + max (f32)
AG1_BN = DC * 128 * ML                # 262144 bf16
AG1_SB = AG1_BN                       # f32[512] as 1024 bf16
AG1_NUL = AG1_SB + 2 * ML
AG1_RMX = AG1_NUL + 2 * ML
AG1_BLK = AG1_RMX + 2 * 128

_CACHE = {}


def _build_nc():
    import concourse.bass as bass
    import concourse.mybir as mybir
    import concourse.tile as tile
    from concourse import bacc, library_config

    dt = mybir.dt
    AF = mybir.ActivationFunctionType
    OP = mybir.AluOpType
    f32, bf16 = dt.float32, dt.bfloat16
    X = mybir.AxisListType.X

    nc = bacc.Bacc("TRN2", target_bir_lowering=False, debug=False, num_devices=R,
                   num_swdge_queues=1)

    # ---- parameters
    ptr_t = nc.declare_dram_parameter("ptr_t", [DC, 128, NL], f32, isOutput=False)
    nu_t = nc.declare_dram_parameter("nu_t", [DC, 128, ML], f32, isOutput=False)
    nu_lg = nc.declare_dram_parameter("nu_lg", [4, 128, K], f32, isOutput=False)
    w_in = nc.declare_dram_parameter("w", [WC, 128, D], f32, isOutput=False)
    ne_idx = nc.declare_dram_parameter("ne_idx", [128, M // 16], dt.int16, isOutput=False)
    pe_idx = nc.declare_dram_parameter("pe_idx", [128, NL // 16], dt.int16, isOutput=False)
    iota_f = nc.declare_dram_parameter("iota_f", [128, K], f32, isOutput=False)
    lbl_f = nc.declare_dram_parameter("lbl_f", [128, 4], f32, isOutput=False)
    out_p = nc.declare_dram_parameter("out", [4101], f32, isOutput=True)

    # ---- internal DRAM
    wn_dram = nc.dram_tensor("wn_dram", [WC * 128, D], bf16)
    ag1_in = nc.dram_tensor("ag1_in", [AG1_BLK], bf16)
    ag1_out = nc.dram_tensor("ag1_out", [R, AG1_BLK], bf16, addr_space="Shared")
    ar_in = nc.dram_tensor("ar_in", [M + 264], f32)
    ar_out = nc.dram_tensor("ar_out", [M + 264], f32, addr_space="Shared")
    fin_dram = nc.dram_tensor("fin_dram", [256], f32)

    rg = [list(range(R))]

    with tile.TileContext(nc, num_cores=R) as tc:
        with (
            tc.tile_pool(name="per", bufs=1) as per,
            tc.tile_pool(name="scr", bufs=1) as scr,
            tc.tile_pool(name="sqp_", bufs=2) as sqpool,
            tc.tile_pool(name="prowp", bufs=2) as prowp,
            tc.tile_pool(name="ez", bufs=1) as ezp,
            tc.tile_pool(name="mskp", bufs=2) as mskp,
            tc.tile_pool(name="mm", bufs=5, space="PSUM") as mmp,
            tc.tile_pool(name="aux", bufs=3, space="PSUM") as aux,
        ):
            nc.gpsimd.load_library(library_config.mlp)

            # constants
            ones_col_h = per.tile([128, 1], bf16, tag="ones_col_h")
            nc.vector.memset(ones_col_h[:], 1.0)
            ones_row_h = per.tile([1, 128], bf16, tag="ones_row_h")
            nc.vector.memset(ones_row_h[:], 1.0)
            cmat = per.tile([3, 128], bf16, tag="cmat")
            nc.vector.memset(cmat[:], 1.0)
            crhs = per.tile([3, M], bf16, tag="crhs")
            nc.vector.memset(crhs[:], 1.0)

# CUTE
            # ---------------- E. CE losses of the nu shard (pick via iota mask)
            lg_s = per.tile([128, 4, K], f32, tag="lg16")
            nc.sync.dma_start(lg_s[:], nu_lg.rearrange("c p k -> p c k"))
            iota_s = per.tile([128, K], f32, tag="iota_s")
            nc.sync.dma_start(iota_s[:], iota_f[:])
            lbl_s = per.tile([128, 4], f32, tag="lbl_s")
            nc.sync.dma_start(lbl_s[:], lbl_f[:])
            sume = per.tile([128, 4], f32, tag="sume")
            pick = per.tile([128, 4], f32, tag="pick")
            for cc in range(4):
                ez = ezp.tile([128, K], f32, tag="ez")
                nc.scalar.activation(ez[:], lg_s[:, cc], AF.Exp,
                                     accum_out=sume[:, cc:cc + 1])
                msk = mskp.tile([128, K], f32, tag="msk")
                nc.vector.tensor_scalar(msk[:], iota_s[:], lbl_s[:, cc:cc + 1], None,
                                        OP.is_equal)
                nc.vector.tensor_tensor(msk[:], msk[:], lg_s[:, cc], op=OP.mult)
                nc.vector.reduce_sum(pick[:, cc:cc + 1], msk[:], axis=X)
            lse = per.tile([128, 4], f32, tag="lse")
            nc.scalar.activation(lse[:], sume[:], AF.Ln)
            nul4 = per.tile([128, 4], f32, tag="nul4")
            nc.vector.tensor_tensor(nul4[:], lse[:], pick[:], op=OP.subtract)
            rmax = per.tile([128, 1], f32, tag="rmax")
            nc.vector.reduce_max(rmax[:], nul4[:], axis=X)
# CUTE2
            nc.sync.dma_start(
                ag1_in[AG1_NUL:AG1_NUL + 2 * ML].bitcast(f32).rearrange("(c p) -> p c", p=128),
                nul4[:])
            nc.sync.dma_start(
                ag1_in[AG1_RMX:AG1_RMX + 2 * 128].bitcast(f32)[:, None], rmax[:])

# CUTA
            # ---------------- A. normalize classifier rows -> wn (bf16) -> DRAM
            w_s = per.tile([128, WC, D], f32, tag="big16")
            nc.sync.dma_start(w_s[:], w_in.rearrange("c p d -> p c d"))
            w_n2 = per.tile([128, WC], f32, tag="w_n2")
            for wc in range(WC):
                sq = sqpool.tile([128, D], f32, tag="sq")
                nc.scalar.activation(sq[:], w_s[:, wc], AF.Square,
                                     accum_out=w_n2[:, wc:wc + 1])
            w_nrm = per.tile([128, WC], f32, tag="w_nrm")
            nc.scalar.activation(w_nrm[:], w_n2[:], AF.Sqrt)
            w_rn = per.tile([128, WC], f32, tag="w_rn")
            nc.vector.reciprocal(w_rn[:], w_nrm[:])
            wn_s = per.tile([128, 2, M], bf16, tag="bhalf", name="wn_s")[:, 0, :].rearrange("p (c d) -> p c d", d=D)
            for wc in range(WC):
                nc.vector.tensor_scalar_mul(wn_s[:, wc], w_s[:, wc], w_rn[:, wc:wc + 1])
            nc.sync.dma_start(wn_dram.rearrange("(c p) d -> p c d", p=128), wn_s[:])

            # ---------------- C. normalize this core's nu feature shard
            nu_ts = per.tile([128, DC, M], bf16, tag="big32", name="nu_ts")[:, :, 0:2 * ML].bitcast(f32)
            nc.sync.dma_start(nu_ts[:], nu_t.rearrange("c p j -> p c j"))
            sqn = [sqpool.tile([128, D], bf16, tag="sqh", name=f"sqn{_k}")
                   for _k in range(DC)]
            for kc in range(DC):
                nc.scalar.activation(sqn[kc][:, 0:ML], nu_ts[:, kc], AF.Square)
            nn2_ps = aux.tile([1, ML], f32, tag="aux")
            for kc in range(DC):
                nc.tensor.matmul(nn2_ps[:], lhsT=ones_col_h[:], rhs=sqn[kc][:, 0:ML],
                                 start=(kc == 0), stop=(kc == DC - 1))
            nn = scr.tile([1, ML], f32, tag="nn")
            nc.scalar.activation(nn[:], nn2_ps[:], AF.Sqrt)
            rn = scr.tile([1, ML], bf16, tag="rn")
            with nc.allow_low_precision(reason="bf16 col-norm reciprocal"):
                nc.vector.reciprocal(rn[:], nn[:])
            rnB_ps = aux.tile([128, ML], f32, tag="aux")
            nc.tensor.matmul(rnB_ps[:], lhsT=ones_row_h[:], rhs=rn[:],
                             start=True, stop=True)
            bn_sh = per.tile([128, DC, ML], bf16, tag="bn_sh")
            sb4 = per.tile([128, DC], f32, tag="sb4")
            for kc in range(DC):
                nc.vector.tensor_tensor(bn_sh[:, kc], nu_ts[:, kc], rnB_ps[:], op=OP.mult)
                nc.vector.reduce_sum(sb4[:, kc:kc + 1], bn_sh[:, kc], axis=X)
            # AG1: bn shard + sb partials
            nc.sync.dma_start(
                ag1_in[0:AG1_BN].rearrange("(c p j) -> p c j", p=128, j=ML), bn_sh[:])
            nc.sync.dma_start(
                ag1_in[AG1_SB:AG1_SB + 2 * ML].bitcast(f32).rearrange("(c p) -> p c", p=128),
                sb4[:])
            nc.gpsimd.collective_compute(
                "AllGather", OP.bypass, replica_groups=rg,
                ins=[ag1_in[:].opt()], outs=[ag1_out[:].opt()],
            )

# CUTB
            # ---------------- B. gather label embeddings (transposed, bf16)
            ne_idx_s = per.tile([128, M // 16], dt.int16, tag="ne_idx_s")
            nc.sync.dma_start(ne_idx_s[:], ne_idx[:])
            pe_idx_s = per.tile([128, NL // 16], dt.int16, tag="pe_idx_s")
            nc.sync.dma_start(pe_idx_s[:], pe_idx[:])
            pr1 = scr.tile([1, M // 16], dt.int16, tag="pr1")
            nc.gpsimd.tensor_copy(pr1[:], ne_idx_s[0:1, :])
            pr2 = scr.tile([1, NL // 16], dt.int16, tag="pr2")
            nc.gpsimd.tensor_copy(pr2[:], pe_idx_s[0:1, :])
            pr3 = scr.tile([1, 16], bf16, tag="pr3")
            nc.gpsimd.dma_start(pr3[:], wn_dram[0:1, 0:16])
            ne_q = [per.tile([128, DC, 512], bf16, tag=f"ne_q{_q}", name=f"ne_q{_q}")
                    for _q in range(8)]
            for q in range(8):
                nc.gpsimd.dma_gather(out_ap=ne_q[q][:], in_ap=wn_dram[:, :],
                                     idxs_ap=ne_idx_s[:, q * 32:(q + 1) * 32],
                                     num_idxs=512, num_idxs_reg=512,
                                     elem_size=D, transpose=True,
                                     queue_num=0)
            pe_s = per.tile([128, DC, NL], bf16, tag="pe_s")
            nc.gpsimd.dma_gather(out_ap=pe_s[:], in_ap=wn_dram[:, :],
                                 idxs_ap=pe_idx_s[:], num_idxs=NL, num_idxs_reg=NL,
                                 elem_size=D, transpose=True, queue_num=0)
            snbh = per.tile([128, DC, 8], f32, tag="snbh")
            spa4 = per.tile([128, DC], f32, tag="spa4")
            for kc in range(DC):
                for q in range(8):
                    nc.vector.reduce_sum(snbh[:, kc, q:q + 1], ne_q[q][:, kc], axis=X)
                nc.vector.reduce_sum(spa4[:, kc:kc + 1], pe_s[:, kc], axis=X)
            snb4 = per.tile([128, DC], f32, tag="snb4")
            nc.vector.reduce_sum(snb4[:], snbh[:], axis=X)

# CUTD
            # ---------------- D. normalize ptr feature shard
            ptr_ts = per.tile([128, DC, NL], f32, tag="ptr_ts")
            nc.sync.dma_start(ptr_ts[:], ptr_t.rearrange("c p i -> p c i"))
            sqp = [sqpool.tile([128, D], bf16, tag="sqh", name=f"sqp{_k}")
                   for _k in range(DC)]
            for kc in range(DC):
                nc.scalar.activation(sqp[kc][:, 0:NL], ptr_ts[:, kc], AF.Square)
            pn2_ps = aux.tile([128, ML], f32, tag="aux", name="pn2_ps")[0:1, 0:NL]
            for kc in range(DC):
                nc.tensor.matmul(pn2_ps[:], lhsT=ones_col_h[:], rhs=sqp[kc][:, 0:NL],
                                 start=(kc == 0), stop=(kc == DC - 1))
            pn = scr.tile([1, NL], f32, tag="pn")
            nc.scalar.activation(pn[:], pn2_ps[:], AF.Sqrt)
            rp = scr.tile([1, NL], bf16, tag="rp")
            with nc.allow_low_precision(reason="bf16 col-norm reciprocal"):
                nc.vector.reciprocal(rp[:], pn[:])
            rpB_ps = aux.tile([128, ML], f32, tag="aux", name="rpB_ps")[:, 0:NL]
            nc.tensor.matmul(rpB_ps[:], lhsT=ones_row_h[:], rhs=rp[:],
                             start=True, stop=True)
            an_ts = per.tile([128, DC, NL], bf16, tag="an_ts")
            sa4 = per.tile([128, DC], f32, tag="sa4")
            for kc in range(DC):
                nc.vector.tensor_tensor(an_ts[:, kc], ptr_ts[:, kc], rpB_ps[:], op=OP.mult)
                nc.vector.reduce_sum(sa4[:, kc:kc + 1], an_ts[:, kc], axis=X)

# CUTG
            # ---------------- G. unpack gathered data
            bn_full = per.tile([128, DC, M], bf16, tag="big32", name="bn_full")
            for r in range(R):
                nc.sync.dma_start(
                    bn_full[:, :, r * ML:(r + 1) * ML],
                    ag1_out[r, 0:AG1_BN].rearrange("(c p j) -> p c j", p=128, j=ML))
            sb8 = per.tile([128, DC, R], f32, tag="sb8")
            for r in range(R):
                nc.sync.dma_start(
                    sb8[:, :, r:r + 1],
                    ag1_out[r, AG1_SB:AG1_SB + 2 * ML].bitcast(f32).rearrange(
                        "(c p) -> p c", p=128)[:, :, None])
            sb_all = per.tile([128, DC], f32, tag="sb_all")
            nc.vector.reduce_sum(sb_all[:], sb8[:], axis=X)

            nul_rs = per.tile([128, M // 128], f32, tag="nul_rs")
            for r in range(R):
                nc.sync.dma_start(
                    nul_rs[16 * r:16 * (r + 1), :],
                    ag1_out[r, AG1_NUL:AG1_NUL + 2 * ML].bitcast(f32).rearrange(
                        "(p c) -> p c", p=16))
            mx_s = per.tile([1, 1024], f32, tag="mx_s")
            for r in range(R):
                nc.sync.dma_start(
                    mx_s[:, 128 * r:128 * (r + 1)],
                    ag1_out[r, AG1_RMX:AG1_RMX + 2 * 128].bitcast(f32)[None, :])

            numax = scr.tile([1, 1], f32, tag="numax")
            nc.vector.reduce_max(numax[:], mx_s[:], axis=X)
            c2 = scr.tile([1, 1], f32, tag="c2")
            nc.vector.tensor_scalar(c2[:], numax[:], -10.0, -2.0, OP.mult, OP.add)
            c2b = scr.tile([1, 128], bf16, tag="c2b")
            nc.vector.tensor_copy(c2b[:], c2[:].to_broadcast([1, 128]))
            nc.sync.dma_start(cmat[2:3, :], c2b[:])
            # hi/lo split of 10*nu_l into crhs rows 0/1
            t10 = scr.tile([128, 32], f32, tag="t10")
            nc.vector.tensor_scalar_mul(t10[:], nul_rs[:], 10.0)
            hi_rs = scr.tile([128, 32], bf16, tag="hi_rs")
            nc.vector.tensor_copy(hi_rs[:], t10[:])
            hif = scr.tile([128, 32], f32, tag="hif")
            nc.vector.tensor_copy(hif[:], hi_rs[:])
            lo_rs = scr.tile([128, 32], bf16, tag="lo_rs")
            nc.vector.tensor_tensor(lo_rs[:], t10[:], hif[:], op=OP.subtract)
            nc.sync.dma_start(crhs[0:1, :], hi_rs[:])
            nc.sync.dma_start(crhs[1:2, :], lo_rs[:])

            # dot-product partials for the C means (ride the final AllReduce)
            dred2 = per.tile([128, 2], f32, tag="dred2")
            dts = scr.tile([128, DC], f32, tag="dts")
            for ci, (a, b) in enumerate(((sa4, sb_all), (spa4, snb4))):
                nc.vector.tensor_tensor(dts[:], a[:], b[:], op=OP.mult)
                nc.vector.reduce_sum(dred2[:, ci:ci + 1], dts[:], axis=X)
            nc.sync.dma_start(ar_in[M:M + 256].rearrange("(c p) -> p c", p=128), dred2[:])

# CUTH
            # ---------------- H. fused GEMMs + exp epilogue
            E_s = per.tile([128, 2, M], bf16, tag="bhalf", name="E_s")
            racc = per.tile([128, 16], f32, tag="racc")
            tiles = [(it, jc) for it in range(2) for jc in range(8)]
            psums = {}

            def ne_half(t):
                it, jc = t
                pt = mmp.tile([128, 512], f32, tag="mmps", name=f"mmps_{t[0]}_{t[1]}")
                psums[t] = pt
                for kc in range(DC):
                    nc.tensor.matmul(pt[:], lhsT=pe_s[:, kc, it * 128:(it + 1) * 128],
                                     rhs=ne_q[jc][:, kc],
                                     start=(kc == 0), stop=False)

            def finish_tile(t):
                it, jc = t
                pt = psums.pop(t)
                for kc in range(DC):
                    nc.tensor.matmul(pt[:], lhsT=an_ts[:, kc, it * 128:(it + 1) * 128],
                                     rhs=bn_full[:, kc, jc * 512:(jc + 1) * 512],
                                     start=False, stop=False)
                nc.tensor.matmul(pt[:], lhsT=cmat[:],
                                 rhs=crhs[:, jc * 512:(jc + 1) * 512],
                                 start=False, stop=True)
                ti = it * 8 + jc
                nc.scalar.activation(E_s[:, it, jc * 512:(jc + 1) * 512], pt[:],
                                     AF.Exp, scale=ACT_SCALE,
                                     accum_out=racc[:, ti:ti + 1])

            for t in tiles[0:5]:
                ne_half(t)
            for i, t in enumerate(tiles):
                finish_tile(t)
                if i + 5 < len(tiles):
                    ne_half(tiles[i + 5])

            vb = per.tile([128, 2], bf16, tag="vb")
            for it in range(2):
                s1 = scr.tile([128, 1], f32, tag="s1")
                nc.vector.reduce_sum(s1[:], racc[:, it * 8:(it + 1) * 8], axis=X)
                s2 = scr.tile([128, 1], f32, tag="s2")
                nc.vector.tensor_scalar_mul(s2[:], s1[:], float(N))
                s3 = scr.tile([128, 1], bf16, tag="s3")
                with nc.allow_low_precision(reason="bf16 softmax scale"):
                    nc.vector.reciprocal(s3[:], s2[:])
                nc.vector.tensor_copy(vb[:, it:it + 1], s3[:])

            for jc in range(8):
                pps_t = aux.tile([128, ML], f32, tag="aux", name=f"pps{jc}")[0:1, :]
                for it in range(2):
                    nc.tensor.matmul(pps_t[:], lhsT=vb[:, it:it + 1],
                                     rhs=E_s[:, it, jc * 512:(jc + 1) * 512],
                                     start=(it == 0), stop=(it == 1))
                prow = prowp.tile([1, 512], f32, tag="prow", name=f"prow{jc}")
                nc.scalar.copy(prow[:], pps_t[:])
                nc.sync.dma_start(ar_in[jc * 512:(jc + 1) * 512][None, :], prow[:])

# CUTI
            # ---------------- I. AllReduce (probs + dot partials)
            nc.gpsimd.collective_compute(
                "AllReduce", OP.add, replica_groups=rg,
                ins=[ar_in[:].opt()], outs=[ar_out[:].opt()],
            )

# CUTJ
            # ---------------- J. final assembly (identical on all cores)
            nc.sync.dma_start(out_p[0:M], ar_out[0:M])
            pr_s = per.tile([128, 32], f32, tag="pr_s")
            nc.sync.dma_start(pr_s[:], ar_out[0:M].rearrange("(p c) -> p c", p=128))
            dd = scr.tile([128, 32], f32, tag="dd")
            ddr = scr.tile([128, 1], f32, tag="ddr")
            nc.vector.tensor_tensor(dd[:], pr_s[:], nul_rs[:], op=OP.mult)
            nc.vector.reduce_sum(ddr[:], dd[:], axis=X)
            nc.sync.dma_start(fin_dram[0:128][:, None], ddr[:])
            drow = scr.tile([1, 128], f32, tag="drow")
            nc.sync.dma_start(drow[:], fin_dram[0:128][None, :])
            dro_s = scr.tile([1, 1], f32, tag="dro_s")
            nc.vector.reduce_sum(dro_s[:], drow[:], axis=X)
            nc.sync.dma_start(out_p[4096:4097][None, :], dro_s[:])
            nc.sync.dma_start(out_p[4097:4098][None, :], dro_s[:])

            dxyrow = scr.tile([1, 2, 128], f32, tag="dxyrow")
            nc.sync.dma_start(
                dxyrow[:],
                ar_out[M:M + 256].rearrange("(c p) -> p c", p=128).rearrange(
                    "p c -> c p")[None])
            dxy = scr.tile([1, 2], f32, tag="dxy")
            nc.vector.reduce_sum(dxy[:], dxyrow[:], axis=X)
            dsum = scr.tile([1, 1], f32, tag="dsum")
            nc.vector.reduce_sum(dsum[:], dxy[:], axis=X)
            mc = scr.tile([1, 1], f32, tag="mc")
            nc.vector.tensor_scalar(mc[:], dsum[:], -1.0 / (N * M), 2.0, OP.mult, OP.add)
            nc.sync.dma_start(out_p[4098:4099][None, :], mc[:])
            mcx = scr.tile([1, 1], f32, tag="mcx")
            nc.vector.tensor_scalar(mcx[:], dxy[:, 0:1], -1.0 / (N * M), 1.0, OP.mult, OP.add)
            nc.sync.dma_start(out_p[4099:4100][None, :], mcx[:])
            mcy = scr.tile([1, 1], f32, tag="mcy")
            nc.vector.tensor_scalar(mcy[:], dxy[:, 1:2], -1.0 / (N * M), 1.0, OP.mult, OP.add)
            nc.sync.dma_start(out_p[4100:4101][None, :], mcy[:])

    nc.compile()
    return nc


def _wrap16(idx):
    n = idx.shape[0]
    a = np.ascontiguousarray(idx.reshape(n // 16, 16).T.astype(np.int16))
    return np.ascontiguousarray(np.tile(a, (8, 1)))  # [128, n/16]


def make_in_maps(ptr_features, ptr_labels, nu_features, nu_logits, nu_labels,
                 classifier_weights):
    w_pad = np.concatenate(
        [np.asarray(classifier_weights, np.float32),
         np.ones((WC * 128 - K, D), np.float32)]).reshape(WC, 128, D)
    ne = _wrap16(np.asarray(nu_labels).astype(np.int64))
    iota = np.broadcast_to(np.arange(K, dtype=np.float32), (128, K)).copy()
    in_maps = []
    for c in range(R):
        p_sh = np.asarray(ptr_features[c * NL:(c + 1) * NL], np.float32)
        nu_sh = np.asarray(nu_features[c * ML:(c + 1) * ML], np.float32)
        lg_sh = np.ascontiguousarray(
            np.asarray(nu_logits[c * ML:(c + 1) * ML], np.float32)).reshape(4, 128, K)
        lbl = np.asarray(nu_labels[c * ML:(c + 1) * ML]).astype(np.int64)
        lblf = np.ascontiguousarray(lbl.reshape(4, 128).T).astype(np.float32)
        in_maps.append({
            "ptr_t": np.ascontiguousarray(p_sh.T).reshape(DC, 128, NL),
            "nu_t": np.ascontiguousarray(nu_sh.T).reshape(DC, 128, ML),
            "nu_lg": lg_sh,
            "w": np.ascontiguousarray(w_pad),
            "ne_idx": ne,
            "pe_idx": _wrap16(np.asarray(ptr_labels[c * NL:(c + 1) * NL]).astype(np.int64)),
            "iota_f": iota,
            "lbl_f": lblf,
        })
    return in_maps


def kernel(ptr_features, ptr_logits, ptr_labels, nu_features, nu_logits,
           nu_labels, classifier_weights):
    from concourse.bass_utils import run_bass_kernel_spmd

    if "nc" not in _CACHE:
        _CACHE["nc"] = _build_nc()
    nc = _CACHE["nc"]
    in_maps = make_in_maps(ptr_features, ptr_labels, nu_features, nu_logits,
                           nu_labels, classifier_weights)
    res = run_bass_kernel_spmd(nc, in_maps, core_ids=list(range(R)))
    o = np.asarray(res.results[0]["out"], np.float32)
    probs = o[0:M].copy()
    return (np.float32(o[4096]), np.float32(o[4097]), probs,
            np.float32(o[4098]), np.float32(o[4099]), np.float32(o[4100]))
